# revision 19
# baseline (speedup 1.0000x reference)
"""DMSAD loss kernel for Trainium2 (8 NeuronCores, data-parallel over batch).

Computes mean over B rows of:
    dist_i = max(min_j ||x_i - c_j||^2, 0)
    loss_i = dist_i                 if st_i == 0
             dist_i + EPS           if st_i == 1
             1 / (dist_i + EPS)     if st_i == -1

Strategy per core (B_SH = 16384 rows, D = 256, C = 128):
  - DMA-cast x fp32 -> bf16 into SBUF (HBM traffic stays fp32; this is the
    roofline driver at ~47us/core).
  - PE-transpose 128x128 bf16 chunks of x so the contraction dim D is on
    partitions; bf16 matmuls vs (-2c)^T accumulate G = -2 x.c^T in PSUM fp32;
    a K=2 matmul of ones vs [bf16_hi(c2); bf16_lo(c2)] folds + c2_j into the
    same PSUM accumulation (hi/lo split keeps c2 at fp32-ish precision).
  - ACT: Square activation with accum_out gives x2 row sums.
  - DVE: PSUM->SBUF copy of transposed x, batched min-reduce over centers.
  - Endgame on [128, 128] tiles: dist = relu(x2 + min), transpose once so the
    layout matches naturally-loaded semi_target, then the select/reciprocal
    arithmetic, row sums, and a ones-matmul partition reduction to one scalar.
Host sums the 8 per-core partial sums and divides by global B.
"""

from contextlib import ExitStack

import numpy as np

import concourse.bass as bass
import concourse.tile as tile
from concourse import bacc, mybir
from concourse.bass_utils import run_bass_kernel_spmd
from concourse.masks import make_identity

N_CORES = 8
B = 131072
D = 256
C = 128
P = 128
B_SH = B // N_CORES          # 16384 rows per core
NT = B_SH // P               # 128 b-tiles of 128 rows
PSUM_GROUP = 4               # b-tiles per PSUM batch (one G bank)
DMA_GROUP = 8                # b-tiles per input DMA (1 MiB fp32 reads)
ETA = 1.0
EPS = 1e-6

# engine balancing knobs:
# out of every 10 tiles, this many compute x2 on DVE (tensor_tensor_reduce);
# the rest use ACT (Square + accum_out).  Copy of transposed x PSUM->SBUF:
# fraction of groups handled by ACT (activation Copy) vs DVE tensor_copy.
X2_DVE_OF_10 = 7
COPY_ACT_OF_10 = 10
# bitcast the PSUM->SBUF copy of transposed x to uint32: halves the element
# count ACT streams (ACT has no 2x packing mode; a pure int move is bit-exact)
COPY_BITCAST = True

F32 = mybir.dt.float32
BF16 = mybir.dt.bfloat16
AF = mybir.ActivationFunctionType
ALU = mybir.AluOpType

_cached_nc = {}


def _emit(ctx: ExitStack, tc, x_d, c_d, st_d, out_d, repeat: int = 1,
          hw_loop: int = 1):
    nc = tc.nc

    const = ctx.enter_context(tc.tile_pool(name="const", bufs=1))
    xpool = ctx.enter_context(tc.tile_pool(name="xin", bufs=3))
    xtps = ctx.enter_context(tc.tile_pool(name="xtps", bufs=3, space="PSUM"))
    xtsb = ctx.enter_context(tc.tile_pool(name="xtsb", bufs=2))
    gps = ctx.enter_context(tc.tile_pool(name="gps", bufs=3, space="PSUM"))
    sqsb = ctx.enter_context(tc.tile_pool(name="sqsb", bufs=2))
    scr_ps = ctx.enter_context(tc.tile_pool(name="scrps", bufs=1, space="PSUM"))
    endp = ctx.enter_context(tc.tile_pool(name="endp", bufs=1))

    # ---- one-time prep -------------------------------------------------
    ident_bf = const.tile([P, P], BF16)
    make_identity(nc, ident_bf[:])
    ident_f32 = const.tile([P, P], F32)
    make_identity(nc, ident_f32[:])

    c_sb = const.tile([C, D], F32)
    nc.sync.dma_start(c_sb[:], c_d[:])

    # c2 = rowsum(c^2) as a [128, 1] fp32 column
    c_sq = const.tile([C, D], F32)
    c2col = const.tile([C, 1], F32)
    nc.scalar.activation(c_sq[:], c_sb[:], AF.Square, accum_out=c2col[:])

    # (-2c) in bf16, then its transpose cT [d-chunk partitions, k, centers]
    cm2 = const.tile([C, D], BF16)
    nc.vector.tensor_scalar_mul(cm2[:], c_sb[:], -2.0)
    ct_ps = scr_ps.tile([P, 2, C], BF16, tag="scratch")
    for k in range(2):
        nc.tensor.transpose(ct_ps[:, k, :], cm2[:, k * P:(k + 1) * P], ident_bf[:])
    cT = const.tile([P, 2, C], BF16)
    nc.vector.tensor_copy(cT[:], ct_ps[:])

    # c2 as two bf16 K-rows (hi + lo) so a K=2 ones-matmul adds fp32-accurate c2
    c2t_ps = scr_ps.tile([1, C], F32, tag="scratch")
    nc.tensor.transpose(c2t_ps[:], c2col[:], ident_f32[:])
    c2row_f = const.tile([1, C], F32)
    nc.vector.tensor_copy(c2row_f[:], c2t_ps[:])
    c2rows = const.tile([2, C], BF16)
    nc.vector.tensor_copy(c2rows[0:1, :], c2row_f[:])
    c2hi_f = const.tile([1, C], F32)
    nc.vector.tensor_copy(c2hi_f[:], c2rows[0:1, :])
    c2lo_f = const.tile([1, C], F32)
    nc.vector.tensor_tensor(c2lo_f[:], c2row_f[:], c2hi_f[:], op=ALU.subtract)
    # engines can't write at base partition 1; a casting SBUF->SBUF DMA can
    nc.gpsimd.dma_start(c2rows[1:2, :], c2lo_f[:])

    ones2 = const.tile([2, C], BF16)
    nc.vector.memset(ones2[:], 1.0)
    ones_col = const.tile([P, 1], F32)
    nc.vector.memset(ones_col[:], 1.0)

    # semi_target, naturally laid out: st_sb[q, m] = st[q*128 + m]
    st_sb = const.tile([P, NT], F32)
    nc.sync.dma_start(st_sb[:], st_d.rearrange("(q m) -> q m", q=P))

    # per-b-tile accumulators: column j <-> b-tile j, partition p <-> row in tile
    mw = const.tile([P, NT], F32)
    x2w = const.tile([P, NT], F32)

    # ---- main loop (repeat/hw_loop >1 only for steady-state benchmarking) ----
    from contextlib import nullcontext
    group_idx = 0
    with tc.For_i(0, hw_loop, 1) if hw_loop > 1 else nullcontext():
     for _rep in range(repeat):
      for gd in range(NT // DMA_GROUP):
        x8 = xpool.tile([P, DMA_GROUP, D], BF16)
        src = x_d[gd * DMA_GROUP * P:(gd + 1) * DMA_GROUP * P, :]
        nc.gpsimd.dma_start(x8[:], src.rearrange("(t p) d -> p t d", p=P))

        for gp in range(DMA_GROUP // PSUM_GROUP):
            tiles = [gp * PSUM_GROUP + t for t in range(PSUM_GROUP)]

            xt_ps = xtps.tile([P, PSUM_GROUP, 2, P], BF16)
            for i, t in enumerate(tiles):
                for k in range(2):
                    nc.tensor.transpose(
                        xt_ps[:, i, k, :], x8[:, t, k * P:(k + 1) * P], ident_bf[:]
                    )
            xt_sb = xtsb.tile([P, PSUM_GROUP, 2, P], BF16)
            if COPY_BITCAST:
                cp_src = xt_ps[:].bitcast(mybir.dt.float32)
                cp_dst = xt_sb[:].bitcast(mybir.dt.float32)
            else:
                cp_src, cp_dst = xt_ps[:], xt_sb[:]
            if (group_idx % 10) < COPY_ACT_OF_10:
                nc.scalar.copy(cp_dst, cp_src)
            else:
                nc.vector.tensor_copy(cp_dst, cp_src)

            g_ps = gps.tile([P, PSUM_GROUP, C], F32)
            for i in range(PSUM_GROUP):
                nc.tensor.matmul(
                    g_ps[:, i, :], lhsT=ones2[:], rhs=c2rows[:],
                    start=True, stop=False,
                )
                nc.tensor.matmul(
                    g_ps[:, i, :], lhsT=xt_sb[:, i, 0, :], rhs=cT[:, 0, :],
                    start=False, stop=False,
                )
                nc.tensor.matmul(
                    g_ps[:, i, :], lhsT=xt_sb[:, i, 1, :], rhs=cT[:, 1, :],
                    start=False, stop=True,
                )

            for i, t in enumerate(tiles):
                col = gd * DMA_GROUP + t
                if (col % 10) < X2_DVE_OF_10:
                    sq = sqsb.tile([P, D], F32, tag="sqd")
                    nc.vector.scalar_tensor_tensor(
                        out=sq[:], in0=x8[:, t, :], scalar=1.0, in1=x8[:, t, :],
                        op0=ALU.mult, op1=ALU.mult,
                        accum_out=x2w[:, col:col + 1],
                    )
                else:
                    sq = sqsb.tile([P, D], F32, tag="sqa")
                    nc.scalar.activation(
                        sq[:], x8[:, t, :], AF.Square,
                        accum_out=x2w[:, col:col + 1],
                    )

            col0 = gd * DMA_GROUP + tiles[0]
            nc.vector.tensor_reduce(
                mw[:, col0:col0 + PSUM_GROUP], g_ps[:], axis=mybir.AxisListType.X,
                op=ALU.min,
            )
            group_idx += 1

    # ---- endgame -------------------------------------------------------
    dist = endp.tile([P, NT], F32)
    nc.vector.tensor_tensor(dist[:], x2w[:], mw[:], op=ALU.add)
    nc.vector.tensor_scalar_max(dist[:], dist[:], 0.0)

    dt_ps = scr_ps.tile([P, NT], F32, tag="scratch")
    nc.tensor.transpose(dt_ps[:], dist[:], ident_f32[:])
    dT = endp.tile([P, NT], F32)
    nc.vector.tensor_copy(dT[:], dt_ps[:])

    dp = endp.tile([P, NT], F32)
    nc.vector.tensor_scalar_add(dp[:], dT[:], EPS)
    r = endp.tile([P, NT], F32)
    nc.vector.reciprocal(r[:], dp[:])

    # loss = dT + min(st,0)*(dT - r) + max(st,0)*EPS
    t1 = endp.tile([P, NT], F32)
    nc.vector.tensor_tensor(t1[:], dT[:], r[:], op=ALU.subtract)
    mneg = endp.tile([P, NT], F32)
    nc.vector.tensor_scalar_min(mneg[:], st_sb[:], 0.0)
    t2 = endp.tile([P, NT], F32)
    nc.vector.tensor_tensor(t2[:], mneg[:], t1[:], op=ALU.mult)
    t3 = endp.tile([P, NT], F32)
    nc.vector.tensor_tensor(t3[:], dT[:], t2[:], op=ALU.add)
    epsq = endp.tile([P, NT], F32)
    nc.vector.tensor_scalar(epsq[:], st_sb[:], 0.0, EPS, op0=ALU.max, op1=ALU.mult)
    losses = endp.tile([P, NT], F32)
    nc.vector.tensor_tensor(losses[:], t3[:], epsq[:], op=ALU.add)

    lsum = endp.tile([P, 1], F32)
    nc.vector.tensor_reduce(lsum[:], losses[:], axis=mybir.AxisListType.X, op=ALU.add)
    total_ps = scr_ps.tile([1, 1], F32, tag="scratch")
    nc.tensor.matmul(total_ps[:], lhsT=ones_col[:], rhs=lsum[:])
    total_sb = endp.tile([1, 1], F32)
    nc.vector.tensor_copy(total_sb[:], total_ps[:])
    nc.sync.dma_start(out_d[:], total_sb[:])


def build_nc(repeat: int = 1, hw_loop: int = 1):
    key = (repeat, hw_loop)
    if key in _cached_nc:
        return _cached_nc[key]
    nc = bacc.Bacc(
        "TRN2",
        target_bir_lowering=False,
        debug=False,
        enable_asserts=False,
        num_devices=N_CORES,
    )
    x_d = nc.dram_tensor("x", [B_SH, D], F32, kind="ExternalInput").ap()
    c_d = nc.dram_tensor("c", [C, D], F32, kind="ExternalInput").ap()
    st_d = nc.dram_tensor("st", [B_SH], F32, kind="ExternalInput").ap()
    out_d = nc.dram_tensor("out", [1, 1], F32, kind="ExternalOutput").ap()

    with tile.TileContext(nc) as tc:
        with ExitStack() as ctx:
            _emit(ctx, tc, x_d, c_d, st_d, out_d, repeat=repeat, hw_loop=hw_loop)
    nc.compile()
    _cached_nc[key] = nc
    return nc


def make_in_maps(x, c, stf):
    return [
        {
            "x": np.ascontiguousarray(x[i * B_SH:(i + 1) * B_SH]),
            "c": c,
            "st": np.ascontiguousarray(stf[i * B_SH:(i + 1) * B_SH]),
        }
        for i in range(N_CORES)
    ]


def kernel(**inputs) -> np.ndarray:
    x = np.ascontiguousarray(np.asarray(inputs["input"], dtype=np.float32))
    c = np.ascontiguousarray(np.asarray(inputs["c"], dtype=np.float32))
    stf = np.asarray(inputs["semi_target"]).astype(np.float32)

    nc = build_nc()
    res = run_bass_kernel_spmd(nc, make_in_maps(x, c, stf), list(range(N_CORES)))
    total = sum(float(r["out"][0, 0]) for r in res.results)
    return np.asarray(np.float32(total / B))


# revision 27
# speedup vs baseline: 16.5915x; 16.5915x over previous
"""DMSAD loss kernel for Trainium2 (8 NeuronCores, data-parallel over batch).

Computes mean over B rows of:
    dist_i = max(min_j ||x_i - c_j||^2, 0)
    loss_i = dist_i                 if st_i == 0
             dist_i + EPS           if st_i == 1
             1 / (dist_i + EPS)     if st_i == -1

Strategy per core (B_SH = 16384 rows, D = 256, C = 128):
  - DMA-cast x fp32 -> bf16 into SBUF (HBM traffic stays fp32; this is the
    roofline driver at ~47us/core).
  - PE-transpose 128x128 bf16 chunks of x so the contraction dim D is on
    partitions; bf16 matmuls vs (-2c)^T accumulate G = -2 x.c^T in PSUM fp32;
    a K=2 matmul of ones vs [bf16_hi(c2); bf16_lo(c2)] folds + c2_j into the
    same PSUM accumulation (hi/lo split keeps c2 at fp32-ish precision).
  - ACT: Square activation with accum_out gives x2 row sums.
  - DVE: PSUM->SBUF copy of transposed x, batched min-reduce over centers.
  - Endgame on [128, 128] tiles: dist = relu(x2 + min), transpose once so the
    layout matches naturally-loaded semi_target, then the select/reciprocal
    arithmetic, row sums, and a ones-matmul partition reduction to one scalar.
Host sums the 8 per-core partial sums and divides by global B.
"""

from contextlib import ExitStack

import numpy as np

import concourse.bass as bass
import concourse.tile as tile
from concourse import bacc, mybir
from concourse.bass_utils import run_bass_kernel_spmd
from concourse.masks import make_identity

N_CORES = 8
B = 131072
D = 256
C = 128
P = 128
B_SH = B // N_CORES          # 16384 rows per core
NT = B_SH // P               # 128 b-tiles of 128 rows
PSUM_GROUP = 4               # b-tiles per PSUM batch (one G bank)
DMA_GROUP = 8                # b-tiles per input DMA (1 MiB fp32 reads)
ETA = 1.0
EPS = 1e-6

# engine balancing knobs:
# out of every 10 tiles, this many compute x2 on DVE (scalar_tensor_tensor);
# the rest use ACT (Square + accum_out).  Copy of transposed x PSUM->SBUF:
# fraction of groups handled by ACT (activation Copy) vs DVE tensor_copy.
X2_DVE_OF_10 = 5
COPY_ACT_OF_10 = 10
# bitcast the PSUM->SBUF copy of transposed x to f32 pairs: halves the element
# count ACT streams (ACT has no 2x packing mode; Copy is exact on normals)
COPY_BITCAST = True
# DMA_CAST: cast fp32->bf16 inside the (SWDGE) DMA.  On this hardware that
# path runs at ~10 GB/s (software cast), so default is a plain HWDGE fp32
# load + on-chip cast on DVE (2x mode) instead.
DMA_CAST = False

F32 = mybir.dt.float32
BF16 = mybir.dt.bfloat16
AF = mybir.ActivationFunctionType
ALU = mybir.AluOpType

_cached_nc = {}


def _emit(ctx: ExitStack, tc, x_d, c_d, st_d, out_d, repeat: int = 1,
          hw_loop: int = 1):
    nc = tc.nc

    const = ctx.enter_context(tc.tile_pool(name="const", bufs=1))
    xpool = ctx.enter_context(tc.tile_pool(name="xin", bufs=3))
    xtps = ctx.enter_context(tc.tile_pool(name="xtps", bufs=3, space="PSUM"))
    xtsb = ctx.enter_context(tc.tile_pool(name="xtsb", bufs=2))
    gps = ctx.enter_context(tc.tile_pool(name="gps", bufs=3, space="PSUM"))
    sqsb = ctx.enter_context(tc.tile_pool(name="sqsb", bufs=2))
    scr_ps = ctx.enter_context(tc.tile_pool(name="scrps", bufs=1, space="PSUM"))
    endp = ctx.enter_context(tc.tile_pool(name="endp", bufs=1))

    # ---- one-time prep -------------------------------------------------
    ident_bf = const.tile([P, P], BF16)
    make_identity(nc, ident_bf[:])
    ident_f32 = const.tile([P, P], F32)
    make_identity(nc, ident_f32[:])

    c_sb = const.tile([C, D], F32)
    nc.sync.dma_start(c_sb[:], c_d[:])

    # c2 = rowsum(c^2) as a [128, 1] fp32 column
    c_sq = const.tile([C, D], F32)
    c2col = const.tile([C, 1], F32)
    nc.scalar.activation(c_sq[:], c_sb[:], AF.Square, accum_out=c2col[:])

    # (-2c) in bf16, then its transpose cT [d-chunk partitions, k, centers]
    cm2 = const.tile([C, D], BF16)
    nc.vector.tensor_scalar_mul(cm2[:], c_sb[:], -2.0)
    ct_ps = scr_ps.tile([P, 2, C], BF16, tag="scratch")
    for k in range(2):
        nc.tensor.transpose(ct_ps[:, k, :], cm2[:, k * P:(k + 1) * P], ident_bf[:])
    cT = const.tile([P, 2, C], BF16)
    nc.vector.tensor_copy(cT[:], ct_ps[:])

    # c2 as two bf16 K-rows (hi + lo) so a K=2 ones-matmul adds fp32-accurate c2
    c2t_ps = scr_ps.tile([1, C], F32, tag="scratch")
    nc.tensor.transpose(c2t_ps[:], c2col[:], ident_f32[:])
    c2row_f = const.tile([1, C], F32)
    nc.vector.tensor_copy(c2row_f[:], c2t_ps[:])
    c2rows = const.tile([2, C], BF16)
    nc.vector.tensor_copy(c2rows[0:1, :], c2row_f[:])
    c2hi_f = const.tile([1, C], F32)
    nc.vector.tensor_copy(c2hi_f[:], c2rows[0:1, :])
    c2lo_f = const.tile([1, C], F32)
    nc.vector.tensor_tensor(c2lo_f[:], c2row_f[:], c2hi_f[:], op=ALU.subtract)
    # engines can't write at base partition 1; a casting SBUF->SBUF DMA can
    nc.gpsimd.dma_start(c2rows[1:2, :], c2lo_f[:])

    ones2 = const.tile([2, C], BF16)
    nc.vector.memset(ones2[:], 1.0)
    ones_col = const.tile([P, 1], F32)
    nc.vector.memset(ones_col[:], 1.0)

    # semi_target, laid out to match the x row mapping below:
    # batch row i = g*(DMA_GROUP*P) + p*DMA_GROUP + t  lives at
    # st_sb[p, g*DMA_GROUP + t]
    st_sb = const.tile([P, NT], F32)
    nc.sync.dma_start(
        st_sb[:].rearrange("p (g t) -> p g t", t=DMA_GROUP),
        st_d.rearrange("(g p t) -> p g t", p=P, t=DMA_GROUP),
    )

    # per-b-tile accumulators: column j <-> b-tile j, partition p <-> row in tile
    mw = const.tile([P, NT], F32)
    x2w = const.tile([P, NT], F32)

    # ---- main loop (repeat/hw_loop >1 only for steady-state benchmarking) ----
    from contextlib import nullcontext
    group_idx = 0
    with tc.For_i(0, hw_loop, 1) if hw_loop > 1 else nullcontext():
     for _rep in range(repeat):
      for gd in range(NT // DMA_GROUP):
        src = x_d[gd * DMA_GROUP * P:(gd + 1) * DMA_GROUP * P, :]
        # row (p, t) of this group = batch gd*1024 + p*8 + t: each partition
        # reads one contiguous 8 KiB run per DMA
        src = src.rearrange("(p t) d -> p t d", t=DMA_GROUP)
        if DMA_CAST:
            x8 = xpool.tile([P, DMA_GROUP, D], BF16, tag="xb")
            nc.gpsimd.dma_start(x8[:], src)
        else:
            xf8 = xpool.tile([P, DMA_GROUP, D], F32, tag="xf")
            nc.sync.dma_start(xf8[:], src)
            x8 = xpool.tile([P, DMA_GROUP, D], BF16, tag="xb")
            nc.vector.tensor_copy(x8[:], xf8[:])

        for gp in range(DMA_GROUP // PSUM_GROUP):
            tiles = [gp * PSUM_GROUP + t for t in range(PSUM_GROUP)]

            xt_ps = xtps.tile([P, PSUM_GROUP, 2, P], BF16)
            for i, t in enumerate(tiles):
                for k in range(2):
                    nc.tensor.transpose(
                        xt_ps[:, i, k, :], x8[:, t, k * P:(k + 1) * P], ident_bf[:]
                    )
            xt_sb = xtsb.tile([P, PSUM_GROUP, 2, P], BF16)
            if COPY_BITCAST:
                cp_src = xt_ps[:].bitcast(mybir.dt.float32)
                cp_dst = xt_sb[:].bitcast(mybir.dt.float32)
            else:
                cp_src, cp_dst = xt_ps[:], xt_sb[:]
            if (group_idx % 10) < COPY_ACT_OF_10:
                nc.scalar.copy(cp_dst, cp_src)
            else:
                nc.vector.tensor_copy(cp_dst, cp_src)

            g_ps = gps.tile([P, PSUM_GROUP, C], F32)
            for i in range(PSUM_GROUP):
                nc.tensor.matmul(
                    g_ps[:, i, :], lhsT=ones2[:], rhs=c2rows[:],
                    start=True, stop=False,
                )
                nc.tensor.matmul(
                    g_ps[:, i, :], lhsT=xt_sb[:, i, 0, :], rhs=cT[:, 0, :],
                    start=False, stop=False,
                )
                nc.tensor.matmul(
                    g_ps[:, i, :], lhsT=xt_sb[:, i, 1, :], rhs=cT[:, 1, :],
                    start=False, stop=True,
                )

            for i, t in enumerate(tiles):
                col = gd * DMA_GROUP + t
                if (col % 10) < X2_DVE_OF_10:
                    # bf16 scratch keeps every operand 2-byte -> DVE 2x mode
                    sq = sqsb.tile([P, D], BF16, tag="sqd")
                    nc.vector.scalar_tensor_tensor(
                        out=sq[:], in0=x8[:, t, :], scalar=1.0, in1=x8[:, t, :],
                        op0=ALU.mult, op1=ALU.mult,
                        accum_out=x2w[:, col:col + 1],
                    )
                else:
                    sq = sqsb.tile([P, D], F32, tag="sqa")
                    nc.scalar.activation(
                        sq[:], x8[:, t, :], AF.Square,
                        accum_out=x2w[:, col:col + 1],
                    )

            col0 = gd * DMA_GROUP + tiles[0]
            nc.vector.tensor_reduce(
                mw[:, col0:col0 + PSUM_GROUP], g_ps[:], axis=mybir.AxisListType.X,
                op=ALU.min,
            )
            group_idx += 1

    # ---- endgame -------------------------------------------------------
    dist = endp.tile([P, NT], F32)
    nc.vector.tensor_tensor(dist[:], x2w[:], mw[:], op=ALU.add)
    nc.vector.tensor_scalar_max(dist[:], dist[:], 0.0)
    dT = dist

    dp = endp.tile([P, NT], F32)
    nc.vector.tensor_scalar_add(dp[:], dT[:], EPS)
    r = endp.tile([P, NT], F32)
    nc.vector.reciprocal(r[:], dp[:])

    # loss = dT + min(st,0)*(dT - r) + max(st,0)*EPS
    t1 = endp.tile([P, NT], F32)
    nc.vector.tensor_tensor(t1[:], dT[:], r[:], op=ALU.subtract)
    mneg = endp.tile([P, NT], F32)
    nc.vector.tensor_scalar_min(mneg[:], st_sb[:], 0.0)
    t2 = endp.tile([P, NT], F32)
    nc.vector.tensor_tensor(t2[:], mneg[:], t1[:], op=ALU.mult)
    t3 = endp.tile([P, NT], F32)
    nc.vector.tensor_tensor(t3[:], dT[:], t2[:], op=ALU.add)
    epsq = endp.tile([P, NT], F32)
    nc.vector.tensor_scalar(epsq[:], st_sb[:], 0.0, EPS, op0=ALU.max, op1=ALU.mult)
    losses = endp.tile([P, NT], F32)
    nc.vector.tensor_tensor(losses[:], t3[:], epsq[:], op=ALU.add)

    lsum = endp.tile([P, 1], F32)
    nc.vector.tensor_reduce(lsum[:], losses[:], axis=mybir.AxisListType.X, op=ALU.add)
    total_ps = scr_ps.tile([1, 1], F32, tag="scratch")
    nc.tensor.matmul(total_ps[:], lhsT=ones_col[:], rhs=lsum[:])
    total_sb = endp.tile([1, 1], F32)
    nc.vector.tensor_copy(total_sb[:], total_ps[:])
    nc.sync.dma_start(out_d[:], total_sb[:])


def build_nc(repeat: int = 1, hw_loop: int = 1, internal_x: bool = False):
    key = (repeat, hw_loop, internal_x)
    if key in _cached_nc:
        return _cached_nc[key]
    nc = bacc.Bacc(
        "TRN2",
        target_bir_lowering=False,
        debug=False,
        enable_asserts=False,
        num_devices=N_CORES,
    )
    if internal_x:
        # timing-only builds: x is internal (uninitialized) DRAM so bench
        # calls don't upload 128 MiB; compute timing is data-independent
        x_d = nc.dram_tensor("x", [B_SH, D], F32).ap()
    else:
        x_d = nc.dram_tensor("x", [B_SH, D], F32, kind="ExternalInput").ap()
    c_d = nc.dram_tensor("c", [C, D], F32, kind="ExternalInput").ap()
    st_d = nc.dram_tensor("st", [B_SH], F32, kind="ExternalInput").ap()
    out_d = nc.dram_tensor("out", [1, 1], F32, kind="ExternalOutput").ap()

    with tile.TileContext(nc) as tc:
        with ExitStack() as ctx:
            _emit(ctx, tc, x_d, c_d, st_d, out_d, repeat=repeat, hw_loop=hw_loop)
    nc.compile()
    _cached_nc[key] = nc
    return nc


def make_in_maps(x, c, stf):
    return [
        {
            "x": np.ascontiguousarray(x[i * B_SH:(i + 1) * B_SH]),
            "c": c,
            "st": np.ascontiguousarray(stf[i * B_SH:(i + 1) * B_SH]),
        }
        for i in range(N_CORES)
    ]


def kernel(**inputs) -> np.ndarray:
    x = np.ascontiguousarray(np.asarray(inputs["input"], dtype=np.float32))
    c = np.ascontiguousarray(np.asarray(inputs["c"], dtype=np.float32))
    stf = np.asarray(inputs["semi_target"]).astype(np.float32)

    nc = build_nc()
    res = run_bass_kernel_spmd(nc, make_in_maps(x, c, stf), list(range(N_CORES)))
    total = sum(float(r["out"][0, 0]) for r in res.results)
    return np.asarray(np.float32(total / B))
